# revision 7
# baseline (speedup 1.0000x reference)
"""AdaptiveLocalPooling Trainium2 kernel (8 NeuronCores, batch-sharded).

For each (b, t): gather K=9 neighbor rows X[b, idx[t,k], :], cosine-sim
against X[b, t, :], softmax over K, weighted-pool the neighbors, then mean
over t -> cls [B, 1, C].

Per-core plan (B_local=2, T=4096, C=384, K=9):
  1. Pre-pass (bulk): gpsimd cast-DMAs X f32 -> bf16 straight into a
     resident SBUF table tab_sb [128, 32*768] (partition p holds rows
     t%128==p, tile-major, both batches packed per 1536B row). Sync
     stores each chunk to the packed DRAM gather table as soon as it
     lands. DVE computes query inv-norms from tab_sb in bulk (one
     reciprocal + ACT sqrt at the end) -> resident f32 ninv_sb.
  2. Main loop over 32 tiles of 128 t's:
       - the 9*128 neighbor rows are gathered with FOUR dma_gather calls
         round-robined over 4 SWDGE queues (4 independent descriptor
         rings keep all 16 SDMA engines busy; a single ring runs at only
         ~206 GB/s, 4 rings hit the ~350 GB/s HBM roofline). Calls must
         stay <= 1024 idxs (bigger hangs the ring).
       - queries are read DIRECTLY from tab_sb (no per-tile DMA).
       - dot[p,k,b] via fused scalar_tensor_tensor (mult+mult, accum_out)
         with the query inv-norm folded into the per-partition scalar;
         neighbor sq-norms via a second STT bank over the gathered rows
         (cheaper than gathering stored norms: keeps rows at 1536B).
       - softmax pipeline (all reduction-class outputs -- STT/ACT
         accum_out, reciprocal -- land LATE on HW, so each is read >= ~19
         instructions after production):
           DVE iter g: sinv(g-3) | nrec(g-1)=1/nn | sim2(g-2)=dots*nsqi |
                       dot/nn STTs(g) | ww(g-3)=ee*sinv
           ACT iter m: nsqi(m)=sqrt(nrec) | exp(m) with accum denominators
       - pooled+mean: 18 PE matmuls [1x384] per tile accumulating
         sum_t sum_k w * X[idx] directly into PSUM across the whole
         kernel; epilogue scales by 1/T.
"""

import os
import sys

import numpy as np

for _p in ("/opt/trn_rl_repo", "/root/.axon_site/_ro/trn_rl_repo"):
    if os.path.isdir(_p) and _p not in sys.path:
        sys.path.insert(0, _p)

import concourse.bacc as bacc
import concourse.bass as bass
import concourse.mybir as mybir
from concourse.bass_utils import run_bass_kernel_spmd
from concourse.library_config import mlp

# Problem sizes (hardcoded per spec).
B = 16
T = 4096
C = 384
K = 9
N_CORES = 8
B_LOC = B // N_CORES  # 2

P = 128
NT = T // P  # 32 tiles
ROW = 2 * C  # 768 elems (1536 B) per packed table row
NIDX = K * P  # 1152 gathered rows per tile
IDX_COLS = NIDX // 16  # 72 idx columns per tile in the wrapped layout
NCHUNK = 4  # pre-pass chunks
TPC = NT // NCHUNK  # tiles per chunk
GDEPTH = 8  # G buffer depth
NQ = 4  # SWDGE queues for the gather
QSPANS = ((0, 3), (3, 5), (5, 7), (7, 9))  # group split across queues

FP32 = mybir.dt.float32
BF16 = mybir.dt.bfloat16
I16 = mybir.dt.int16
AX = mybir.AxisListType
OP = mybir.AluOpType
AF = mybir.ActivationFunctionType


def build_kernel(
    n_tiles: int = NT,
    debug: bool = False,
    pre_reps: int = 1,
    main_reps: int = 1,
    ablate: str = "",
) -> bass.Bass:
    # ablate: comma list of {stt, gather, pe} to stub out (timing-only builds)
    t_loc = n_tiles * P
    nch = max(1, n_tiles // TPC)
    nc = bacc.Bacc("TRN2", num_swdge_queues=NQ)

    x_ext = nc.declare_dram_parameter("X", [B_LOC, t_loc, C], FP32, isOutput=False)
    idx_ext = nc.declare_dram_parameter(
        "idx", [P, n_tiles * IDX_COLS], I16, isOutput=False
    )
    out_ext = nc.declare_dram_parameter("out", [B_LOC, C], FP32, isOutput=True)
    table = nc.dram_tensor("table", [t_loc, ROW], BF16)

    from contextlib import ExitStack

    with ExitStack() as ctx:
        e = ctx.enter_context

        idx_sb = e(nc.sbuf_tensor("idx_sb", [P, n_tiles * IDX_COLS], I16))
        tab_sb = e(nc.sbuf_tensor("tab_sb", [P, n_tiles * ROW], BF16))
        nn_all = e(nc.sbuf_tensor("nn_all", [P, 2 * n_tiles], FP32))
        nrec_all = e(nc.sbuf_tensor("nrec_all", [P, 2 * n_tiles], FP32))
        ninv_sb = e(nc.sbuf_tensor("ninv_sb", [P, 2 * n_tiles], FP32))
        # main-loop buffers
        G = [e(nc.sbuf_tensor(f"G{i}", [P, K * ROW], BF16)) for i in range(GDEPTH)]
        scr = e(nc.sbuf_tensor("scr", [P, C], BF16))
        dots = [e(nc.sbuf_tensor(f"dots{i}", [P, 2 * K], FP32)) for i in range(2)]
        nn_g = [e(nc.sbuf_tensor(f"nn_g{i}", [P, 2 * K], FP32)) for i in range(2)]
        nrec_g = [e(nc.sbuf_tensor(f"nrec_g{i}", [P, 2 * K], FP32)) for i in range(2)]
        nsqi = [e(nc.sbuf_tensor(f"nsqi{i}", [P, 2 * K], FP32)) for i in range(2)]
        sim2 = [e(nc.sbuf_tensor(f"sim2_{i}", [P, 2 * K], FP32)) for i in range(2)]
        ee = [e(nc.sbuf_tensor(f"e{i}", [P, 2 * K], FP32)) for i in range(2)]
        ss = [e(nc.sbuf_tensor(f"s{i}_", [P, 2], FP32)) for i in range(2)]
        sinv = e(nc.sbuf_tensor("sinv", [P, 2], FP32))
        dscr = e(nc.sbuf_tensor("dscr", [P, 2], FP32))
        ww = [e(nc.sbuf_tensor(f"w{i}", [P, 2 * K], BF16)) for i in range(4)]
        out_sb = e(nc.sbuf_tensor("out_sb", [33, C], FP32))
        acc = e(nc.psum_tensor("acc", [64, C], FP32))

        xload = e(nc.semaphore("xload"))  # +32 per cast chunk (2 DMAs)
        vchk = e(nc.semaphore("vchk"))  # +1 per DVE STT chunk
        vpre = e(nc.semaphore("vpre"))  # +1 per pre-pass rep (recip done)
        spre = e(nc.semaphore("spre"))  # +1 per pre-pass rep (ACT sqrt done)
        tdone = e(nc.semaphore("tdone"))  # +16 per table store chunk
        isem = e(nc.semaphore("isem"))
        gsem = [e(nc.semaphore(f"gsem{i}")) for i in range(GDEPTH)]
        nrdy = e(nc.semaphore("nrdy"))  # DVE nrec_g done, +1/tile
        srdy = e(nc.semaphore("srdy"))  # ACT nsqi done, +1/tile
        v1 = e(nc.semaphore("v1"))  # DVE sim2 done, +1/tile
        v2 = e(nc.semaphore("v2"))  # DVE ww done, +1/tile
        aexp = e(nc.semaphore("aexp"))  # ACT exp done, +1/tile
        pe_done = e(nc.semaphore("pe_done"))
        vfin = e(nc.semaphore("vfin"))
        osem = e(nc.semaphore("osem"))

        block = e(nc.Block())
        n_main = main_reps * n_tiles

        def tabx(t, b):  # query x slice of tab_sb for tile t, batch b
            return tab_sb[:, t * ROW + b * C : t * ROW + (b + 1) * C]

        def k3(ap):  # [P, 2K] -> [P, K, 2] (k-major pairs)
            return ap.rearrange("p (a b) -> p a b", b=2)

        def bcast2(ap2):  # [P, 2] -> [P, K, 2] with step-0 broadcast over K
            return ap2.rearrange("p (o b) -> p o b", o=1).to_broadcast([P, K, 2])

        @block.sync
        def _(sync: bass.BassEngine):
            # ---- pre-pass: store table chunks as they land ----
            for r in range(pre_reps):
                for c in range(nch):
                    sync.wait_ge(xload, 32 * (r * nch + c + 1))
                    rows = slice(c * TPC * P, (c + 1) * TPC * P)
                    sync.dma_start(
                        out=table[rows, :].rearrange("(g p) r -> p g r", p=P),
                        in_=tab_sb[
                            :, c * TPC * ROW : (c + 1) * TPC * ROW
                        ].rearrange("p (g r) -> p g r", r=ROW),
                    ).then_inc(tdone, 16)
            # ---- epilogue ----
            sync.wait_ge(vfin, 1)
            sync.dma_start(out=out_ext[0:1, :], in_=out_sb[0:1, :]).then_inc(osem, 16)
            sync.dma_start(out=out_ext[1:2, :], in_=out_sb[32:33, :]).then_inc(
                osem, 16
            )
            sync.wait_ge(osem, 32)

        @block.vector
        def _(vector: bass.BassVectorEngine):
            # ---- pre-pass: bulk query norms from tab_sb ----
            for r in range(pre_reps):
                for c in range(nch):
                    vector.wait_ge(xload, 32 * (r * nch + c + 1))
                    for tt in range(TPC):
                        t = c * TPC + tt
                        for b in range(2):
                            st = vector.scalar_tensor_tensor(
                                out=scr[:],
                                in0=tabx(t, b),
                                scalar=1.0,
                                in1=tabx(t, b),
                                op0=OP.mult,
                                op1=OP.mult,
                                accum_out=nn_all[:, 2 * t + b : 2 * t + b + 1],
                            )
                    st.then_inc(vchk, 1)
                # spacers so the last chunk's accum lands before the recip
                for _ in range(3):
                    vector.tensor_copy(out=scr[:], in_=tab_sb[:, 0:C])
                vector.reciprocal(out=nrec_all[:], in_=nn_all[:]).then_inc(vpre, 1)

            # ---- main loop (see module docstring for the phase schedule) ----
            def phase_a1(m):  # sinv = 1/ss for tile m (ss from ACT accum)
                vector.wait_ge(aexp, m + 1)
                vector.reciprocal(out=sinv[:], in_=ss[m % 2][:])

            def phase_nr(m):  # nrec_g = 1/nn_g for tile m
                vector.reciprocal(
                    out=nrec_g[m % 2][:], in_=nn_g[m % 2][:]
                ).then_inc(nrdy, 1)

            def phase_b(m):  # sim2 = dots * nsqi for tile m (ACT sqrt done)
                vector.wait_ge(srdy, m + 1)
                vector.tensor_tensor(
                    out=k3(sim2[m % 2][:]),
                    in0=k3(dots[m % 2][:]),
                    in1=k3(nsqi[m % 2][:]),
                    op=OP.mult,
                ).then_inc(v1, 1)

            def phase_a2(m):  # ww = ee * sinv for tile m
                if m >= 4:
                    vector.wait_ge(pe_done, m - 3)  # ww[m%4] free
                vector.tensor_tensor(
                    out=k3(ww[m % 4][:]),
                    in0=k3(ee[m % 2][:]),
                    in1=bcast2(sinv[:]),
                    op=OP.mult,
                ).then_inc(v2, 1)

            def spacer(n=2):
                for _ in range(n):
                    vector.tensor_copy(out=scr[:], in_=tab_sb[:, 0:C])

            nspans = 2 if "gather" in ablate else len(QSPANS)
            vector.wait_ge(spre, pre_reps)  # ninv_sb resident
            for g in range(n_main):
                t = g % n_tiles
                if g >= 3:
                    phase_a1(g - 3)
                if g >= 2:
                    phase_b(g - 2)
                vector.wait_ge(gsem[g % GDEPTH], 16 * nspans * (g // GDEPTH + 1))
                for k in range(1 if "stt" in ablate else K):
                    for b in range(2):
                        vector.scalar_tensor_tensor(
                            out=scr[:],
                            in0=tabx(t, b),
                            scalar=ninv_sb[:, 2 * t + b : 2 * t + b + 1],
                            in1=G[g % GDEPTH][
                                :, k * ROW + b * C : k * ROW + (b + 1) * C
                            ],
                            op0=OP.mult,
                            op1=OP.mult,
                            accum_out=dots[g % 2][:, k * 2 + b : k * 2 + b + 1],
                        )
                for k in range(1 if "stt" in ablate else K):
                    for b in range(2):
                        vector.scalar_tensor_tensor(
                            out=scr[:],
                            in0=G[g % GDEPTH][
                                :, k * ROW + b * C : k * ROW + (b + 1) * C
                            ],
                            scalar=1.0,
                            in1=G[g % GDEPTH][
                                :, k * ROW + b * C : k * ROW + (b + 1) * C
                            ],
                            op0=OP.mult,
                            op1=OP.mult,
                            accum_out=nn_g[g % 2][:, k * 2 + b : k * 2 + b + 1],
                        )
                if g >= 1:
                    phase_nr(g - 1)  # lag: after this iter's 36 STTs
                if g >= 3:
                    phase_a2(g - 3)
            # tail (in-loop: a1/a2 m<=n-4, nr m<=n-2, b m<=n-3)
            nm = n_main
            spacer(3)
            phase_nr(nm - 1)
            phase_b(nm - 2)
            phase_a1(nm - 3)
            spacer(3)
            phase_a2(nm - 3)
            phase_b(nm - 1)
            phase_a1(nm - 2)
            spacer(3)
            phase_a2(nm - 2)
            phase_a1(nm - 1)
            spacer(3)
            phase_a2(nm - 1)
            # epilogue: PSUM -> SBUF with 1/T scaling
            vector.wait_ge(pe_done, n_main)
            vector.tensor_scalar_mul(
                out=out_sb[0:1, :], in0=acc[0:1, :], scalar1=1.0 / (t_loc * main_reps)
            )
            vector.tensor_scalar_mul(
                out=out_sb[32:33, :],
                in0=acc[32:33, :],
                scalar1=1.0 / (t_loc * main_reps),
            ).then_inc(vfin, 1)

        @block.scalar
        def _(scalar: bass.BassScalarEngine):
            # ---- pre-pass: ninv = sqrt(1/nn) -> resident f32 ----
            for r in range(pre_reps):
                scalar.wait_ge(vpre, r + 1)
                scalar.activation(
                    out=ninv_sb[:], in_=nrec_all[:], func=AF.Sqrt
                ).then_inc(spre, 1)
            # ---- main loop: neighbor rsqrt + exp with fused denominators;
            # the aexp inc rides a trailing dummy op so the accum lands first
            for m in range(n_main):
                scalar.wait_ge(nrdy, m + 1)
                scalar.activation(
                    out=nsqi[m % 2][:], in_=nrec_g[m % 2][:], func=AF.Sqrt
                ).then_inc(srdy, 1)
                scalar.wait_ge(v1, m + 1)
                if m >= 2:
                    scalar.wait_ge(v2, m - 1)  # ee/ss[m%2] free (a2 lag 3)
                s23 = k3(sim2[m % 2][:])
                e3 = k3(ee[m % 2][:])
                for b in range(2):
                    scalar.activation(
                        out=e3[:, :, b : b + 1],
                        in_=s23[:, :, b : b + 1],
                        func=AF.Exp,
                        accum_out=ss[m % 2][:, b : b + 1],
                    )
                scalar.activation(
                    out=dscr[:], in_=sim2[m % 2][:, 0:2], func=AF.Copy
                ).then_inc(aexp, 1)

        @block.tensor
        def _(tensor: bass.BassTensorEngine):
            for g in range(n_main):
                tensor.wait_ge(v2, g + 1)
                for k in range(1 if "pe" in ablate else K):
                    for b in range(2):
                        mm = tensor.matmul(
                            out=acc[32 * b : 32 * b + 1, :],
                            lhsT=ww[g % 4][:, k * 2 + b : k * 2 + b + 1],
                            rhs=G[g % GDEPTH][
                                :, k * ROW + b * C : k * ROW + (b + 1) * C
                            ],
                            start=(g == 0 and k == 0),
                            stop=(g == n_main - 1 and k == K - 1),
                            skip_group_check=True,
                        )
                mm.then_inc(pe_done, 1)

        @block.gpsimd
        def _(gpsimd: bass.BassGpSimd):
            gpsimd.load_library(mlp)
            gpsimd.dma_start(out=idx_sb[:], in_=idx_ext[:]).then_inc(isem, 16)
            # ---- pre-pass: cast-DMA X f32 -> tab_sb bf16, chunked ----
            for r in range(pre_reps):
                for c in range(nch):
                    if r > 0:
                        # previous rep's consumers of this chunk must finish
                        gpsimd.wait_ge(vchk, (r - 1) * nch + c + 1)
                        gpsimd.wait_ge(tdone, 16 * ((r - 1) * nch + c + 1))
                    rows = slice(c * TPC * P, (c + 1) * TPC * P)
                    tv = tab_sb[
                        :, c * TPC * ROW : (c + 1) * TPC * ROW
                    ].rearrange("p (g b c) -> p g b c", b=2, c=C)
                    for b in range(2):
                        gpsimd.dma_start(
                            out=tv[:, :, b, :],
                            in_=x_ext[b, rows, :].rearrange(
                                "(g p) c -> p g c", p=P
                            ),
                        ).then_inc(xload, 16)
            # ---- main loop: gathers (after full table resident in DRAM) ----
            gpsimd.wait_ge(isem, 16)
            gpsimd.wait_ge(tdone, 16 * pre_reps * nch)
            for g in range(n_main):
                t = g % n_tiles
                if g >= GDEPTH:
                    gpsimd.wait_ge(pe_done, g - (GDEPTH - 2))  # G[g%GDEPTH] free
                gv3 = G[g % GDEPTH][:].rearrange("p (g r) -> p g r", r=ROW)
                spans = ((0, 1), (1, 2)) if "gather" in ablate else QSPANS
                for q, (g0, g1) in enumerate(spans):
                    n = (g1 - g0) * P
                    gpsimd.dma_gather(
                        gv3[:, g0:g1, :],
                        table[:],
                        idx_sb[
                            :,
                            t * IDX_COLS + g0 * 8 : t * IDX_COLS + g1 * 8,
                        ],
                        n,
                        n,
                        ROW,
                        single_packet=True,
                        queue_num=q % NQ,
                    ).then_inc(gsem[g % GDEPTH], 16)

    nc.compile()
    return nc


def make_idx_table(neighbor_idx: np.ndarray, n_tiles: int = NT) -> np.ndarray:
    """Host-side index preprocessing into dma_gather's wrapped int16 layout.

    Flat order per tile: i = k*128 + p  ->  neighbor_idx[t0+p, k].
    Wrapped: idx_sb[q, tile*IDX_COLS + c] = flat[c*16 + q%16].
    """
    nb = np.asarray(neighbor_idx).astype(np.int16)  # values < 4096
    cols = np.empty((P, n_tiles * IDX_COLS), dtype=np.int16)
    for t in range(n_tiles):
        flat = nb[t * P : (t + 1) * P, :].T.reshape(-1)  # [K*P], k-major
        wrap = flat.reshape(IDX_COLS, 16).T  # [16, IDX_COLS]
        cols[:, t * IDX_COLS : (t + 1) * IDX_COLS] = np.tile(wrap, (8, 1))
    return cols


_NC_CACHE: dict = {}


def _get_nc():
    if "nc" not in _NC_CACHE:
        _NC_CACHE["nc"] = build_kernel()
    return _NC_CACHE["nc"]


def kernel(X: np.ndarray, neighbor_idx: np.ndarray, **_ignored) -> np.ndarray:
    X = np.asarray(X, dtype=np.float32)
    idx_cols = make_idx_table(neighbor_idx)
    nc = _get_nc()
    core_ids = list(range(N_CORES))
    in_maps = [
        {"X": np.ascontiguousarray(X[i * B_LOC : (i + 1) * B_LOC]), "idx": idx_cols}
        for i in core_ids
    ]
    res = run_bass_kernel_spmd(nc, in_maps, core_ids)
    outs = [res.results[i]["out"] for i in range(N_CORES)]  # each [B_LOC, C]
    full = np.concatenate(outs, axis=0).reshape(B, 1, C).astype(np.float32)
    return full


if __name__ == "__main__":
    rng = np.random.default_rng(0)
    X = rng.standard_normal((B, T, C), dtype=np.float32)
    nb = rng.integers(0, T, size=(T, K)).astype(np.int64)
    out = kernel(X, nb)
    print("out", out.shape, out.dtype, float(np.abs(out).mean()))


# revision 8
# speedup vs baseline: 1.0143x; 1.0143x over previous
"""AdaptiveLocalPooling Trainium2 kernel (8 NeuronCores, batch-sharded).

For each (b, t): gather K=9 neighbor rows X[b, idx[t,k], :], cosine-sim
against X[b, t, :], softmax over K, weighted-pool the neighbors, then mean
over t -> cls [B, 1, C].

Per-core plan (B_local=2, T=4096, C=384, K=9):
  1. Pre-pass (bulk): gpsimd cast-DMAs X f32 -> bf16 straight into a
     resident SBUF table tab_sb [128, 32*768] (partition p holds rows
     t%128==p, tile-major, both batches packed per 1536B row). Sync
     stores each chunk to the packed DRAM gather table as soon as it
     lands. DVE computes query inv-norms from tab_sb in bulk (one
     reciprocal + ACT sqrt at the end) -> resident f32 ninv_sb.
  2. Main loop over 32 tiles of 128 t's:
       - the 9*128 neighbor rows are gathered with FOUR dma_gather calls
         round-robined over 4 SWDGE queues (4 independent descriptor
         rings keep all 16 SDMA engines busy; a single ring runs at only
         ~206 GB/s, 4 rings hit the ~350 GB/s HBM roofline). Calls must
         stay <= 1024 idxs (bigger hangs the ring).
       - queries are read DIRECTLY from tab_sb (no per-tile DMA).
       - dot[p,k,b] via fused scalar_tensor_tensor (mult+mult, accum_out)
         with the query inv-norm folded into the per-partition scalar;
         neighbor sq-norms via a second STT bank over the gathered rows
         (cheaper than gathering stored norms: keeps rows at 1536B).
       - softmax pipeline (all reduction-class outputs -- STT/ACT
         accum_out, reciprocal -- land LATE on HW, so each is read >= ~19
         instructions after production):
           DVE iter g: sinv(g-3) | nrec(g-1)=1/nn | sim2(g-2)=dots*nsqi |
                       dot/nn STTs(g) | ww(g-3)=ee*sinv
           ACT iter m: nsqi(m)=sqrt(nrec) | exp(m) with accum denominators
       - pooled+mean: 18 PE matmuls [1x384] per tile accumulating
         sum_t sum_k w * X[idx] directly into PSUM across the whole
         kernel; epilogue scales by 1/T.
"""

import os
import sys

import numpy as np

for _p in ("/opt/trn_rl_repo", "/root/.axon_site/_ro/trn_rl_repo"):
    if os.path.isdir(_p) and _p not in sys.path:
        sys.path.insert(0, _p)

import concourse.bacc as bacc
import concourse.bass as bass
import concourse.mybir as mybir
from concourse.bass_utils import run_bass_kernel_spmd
from concourse.library_config import mlp

# Problem sizes (hardcoded per spec).
B = 16
T = 4096
C = 384
K = 9
N_CORES = 8
B_LOC = B // N_CORES  # 2

P = 128
NT = T // P  # 32 tiles
ROW = 2 * C  # 768 elems (1536 B) per packed table row
NIDX = K * P  # 1152 gathered rows per tile
IDX_COLS = NIDX // 16  # 72 idx columns per tile in the wrapped layout
NCHUNK = 4  # pre-pass chunks
TPC = NT // NCHUNK  # tiles per chunk
GDEPTH = 9  # G buffer depth
NQ = 4  # SWDGE queues for the gather
QSPANS = ((0, 3), (3, 5), (5, 7), (7, 9))  # group split across queues

FP32 = mybir.dt.float32
BF16 = mybir.dt.bfloat16
I16 = mybir.dt.int16
AX = mybir.AxisListType
OP = mybir.AluOpType
AF = mybir.ActivationFunctionType


def build_kernel(
    n_tiles: int = NT,
    debug: bool = False,
    pre_reps: int = 1,
    main_reps: int = 1,
    ablate: str = "",
) -> bass.Bass:
    # ablate: comma list of {stt, gather, pe} to stub out (timing-only builds)
    t_loc = n_tiles * P
    nch = max(1, n_tiles // TPC)
    nc = bacc.Bacc("TRN2", num_swdge_queues=NQ)

    x_ext = nc.declare_dram_parameter("X", [B_LOC, t_loc, C], FP32, isOutput=False)
    idx_ext = nc.declare_dram_parameter(
        "idx", [P, n_tiles * IDX_COLS], I16, isOutput=False
    )
    out_ext = nc.declare_dram_parameter("out", [B_LOC, C], FP32, isOutput=True)
    table = nc.dram_tensor("table", [t_loc, ROW], BF16)

    from contextlib import ExitStack

    with ExitStack() as ctx:
        e = ctx.enter_context

        idx_sb = e(nc.sbuf_tensor("idx_sb", [P, n_tiles * IDX_COLS], I16))
        tab_sb = e(nc.sbuf_tensor("tab_sb", [P, n_tiles * ROW], BF16))
        nn_all = e(nc.sbuf_tensor("nn_all", [P, 2 * n_tiles], FP32))
        nrec_all = e(nc.sbuf_tensor("nrec_all", [P, 2 * n_tiles], FP32))
        ninv_sb = e(nc.sbuf_tensor("ninv_sb", [P, 2 * n_tiles], FP32))
        # main-loop buffers
        G = [e(nc.sbuf_tensor(f"G{i}", [P, K * ROW], BF16)) for i in range(GDEPTH)]
        scr = e(nc.sbuf_tensor("scr", [P, C], BF16))
        dots = [e(nc.sbuf_tensor(f"dots{i}", [P, 2 * K], FP32)) for i in range(4)]
        nn_g = [e(nc.sbuf_tensor(f"nn_g{i}", [P, 2 * K], FP32)) for i in range(2)]
        nrec_g = [e(nc.sbuf_tensor(f"nrec_g{i}", [P, 2 * K], FP32)) for i in range(2)]
        nsqi = [e(nc.sbuf_tensor(f"nsqi{i}", [P, 2 * K], FP32)) for i in range(2)]
        sim2 = [e(nc.sbuf_tensor(f"sim2_{i}", [P, 2 * K], FP32)) for i in range(2)]
        ee = [e(nc.sbuf_tensor(f"e{i}", [P, 2 * K], FP32)) for i in range(2)]
        ss = [e(nc.sbuf_tensor(f"s{i}_", [P, 2], FP32)) for i in range(2)]
        sinv = e(nc.sbuf_tensor("sinv", [P, 2], FP32))
        dscr = e(nc.sbuf_tensor("dscr", [P, 2], FP32))
        ww = [e(nc.sbuf_tensor(f"w{i}", [P, 2 * K], BF16)) for i in range(4)]
        out_sb = e(nc.sbuf_tensor("out_sb", [33, C], FP32))
        acc = e(nc.psum_tensor("acc", [64, C], FP32))

        xload = e(nc.semaphore("xload"))  # +32 per cast chunk (2 DMAs)
        vchk = e(nc.semaphore("vchk"))  # +1 per DVE STT chunk
        vpre = e(nc.semaphore("vpre"))  # +1 per pre-pass rep (recip done)
        spre = e(nc.semaphore("spre"))  # +1 per pre-pass rep (ACT sqrt done)
        tdone = e(nc.semaphore("tdone"))  # +16 per table store chunk
        isem = e(nc.semaphore("isem"))
        gsem = [e(nc.semaphore(f"gsem{i}")) for i in range(GDEPTH)]
        nrdy = e(nc.semaphore("nrdy"))  # DVE nrec_g done, +1/tile
        srdy = e(nc.semaphore("srdy"))  # ACT nsqi done, +1/tile
        v1 = e(nc.semaphore("v1"))  # DVE sim2 done, +1/tile
        v2 = e(nc.semaphore("v2"))  # DVE ww done, +1/tile
        aexp = e(nc.semaphore("aexp"))  # ACT exp done, +1/tile
        pe_done = e(nc.semaphore("pe_done"))
        vfin = e(nc.semaphore("vfin"))
        osem = e(nc.semaphore("osem"))

        block = e(nc.Block())
        n_main = main_reps * n_tiles

        def tabx(t, b):  # query x slice of tab_sb for tile t, batch b
            return tab_sb[:, t * ROW + b * C : t * ROW + (b + 1) * C]

        def k3(ap):  # [P, 2K] -> [P, K, 2] (k-major pairs)
            return ap.rearrange("p (a b) -> p a b", b=2)

        def bcast2(ap2):  # [P, 2] -> [P, K, 2] with step-0 broadcast over K
            return ap2.rearrange("p (o b) -> p o b", o=1).to_broadcast([P, K, 2])

        @block.sync
        def _(sync: bass.BassEngine):
            # ---- pre-pass: store table chunks as they land ----
            for r in range(pre_reps):
                for c in range(nch):
                    sync.wait_ge(xload, 32 * (r * nch + c + 1))
                    rows = slice(c * TPC * P, (c + 1) * TPC * P)
                    sync.dma_start(
                        out=table[rows, :].rearrange("(g p) r -> p g r", p=P),
                        in_=tab_sb[
                            :, c * TPC * ROW : (c + 1) * TPC * ROW
                        ].rearrange("p (g r) -> p g r", r=ROW),
                    ).then_inc(tdone, 16)
            # ---- epilogue ----
            sync.wait_ge(vfin, 1)
            sync.dma_start(out=out_ext[0:1, :], in_=out_sb[0:1, :]).then_inc(osem, 16)
            sync.dma_start(out=out_ext[1:2, :], in_=out_sb[32:33, :]).then_inc(
                osem, 16
            )
            sync.wait_ge(osem, 32)

        @block.vector
        def _(vector: bass.BassVectorEngine):
            # ---- pre-pass: bulk query norms from tab_sb ----
            for r in range(pre_reps):
                for c in range(nch):
                    vector.wait_ge(xload, 32 * (r * nch + c + 1))
                    for tt in range(TPC):
                        t = c * TPC + tt
                        for b in range(2):
                            st = vector.scalar_tensor_tensor(
                                out=scr[:],
                                in0=tabx(t, b),
                                scalar=1.0,
                                in1=tabx(t, b),
                                op0=OP.mult,
                                op1=OP.mult,
                                accum_out=nn_all[:, 2 * t + b : 2 * t + b + 1],
                            )
                    st.then_inc(vchk, 1)
                # spacers so the last chunk's accum lands before the recip
                for _ in range(3):
                    vector.tensor_copy(out=scr[:], in_=tab_sb[:, 0:C])
                vector.reciprocal(out=nrec_all[:], in_=nn_all[:]).then_inc(vpre, 1)

            # ---- main loop (see module docstring for the phase schedule) ----
            def phase_a1(m):  # sinv = 1/ss for tile m (ss from ACT accum)
                vector.wait_ge(aexp, m + 1)
                vector.reciprocal(out=sinv[:], in_=ss[m % 2][:])

            def phase_nr(m):  # nrec_g = 1/nn_g for tile m
                vector.reciprocal(
                    out=nrec_g[m % 2][:], in_=nn_g[m % 2][:]
                ).then_inc(nrdy, 1)

            def phase_b(m):  # sim2 = dots * nsqi for tile m (ACT sqrt done)
                vector.wait_ge(srdy, m + 1)
                vector.tensor_tensor(
                    out=k3(sim2[m % 2][:]),
                    in0=k3(dots[m % 4][:]),
                    in1=k3(nsqi[m % 2][:]),
                    op=OP.mult,
                ).then_inc(v1, 1)

            def phase_a2(m):  # ww = ee * sinv for tile m
                if m >= 4:
                    vector.wait_ge(pe_done, m - 3)  # ww[m%4] free
                vector.tensor_tensor(
                    out=k3(ww[m % 4][:]),
                    in0=k3(ee[m % 2][:]),
                    in1=bcast2(sinv[:]),
                    op=OP.mult,
                ).then_inc(v2, 1)

            def spacer(n=2):
                for _ in range(n):
                    vector.tensor_copy(out=scr[:], in_=tab_sb[:, 0:C])

            nspans = 2 if "gather" in ablate else len(QSPANS)
            vector.wait_ge(spre, pre_reps)  # ninv_sb resident
            for g in range(n_main):
                t = g % n_tiles
                if g >= 3:
                    phase_b(g - 3)
                if g >= 4:
                    phase_a1(g - 4)
                vector.wait_ge(gsem[g % GDEPTH], 16 * nspans * (g // GDEPTH + 1))
                for k in range(1 if "stt" in ablate else K):
                    for b in range(2):
                        vector.scalar_tensor_tensor(
                            out=scr[:],
                            in0=tabx(t, b),
                            scalar=ninv_sb[:, 2 * t + b : 2 * t + b + 1],
                            in1=G[g % GDEPTH][
                                :, k * ROW + b * C : k * ROW + (b + 1) * C
                            ],
                            op0=OP.mult,
                            op1=OP.mult,
                            accum_out=dots[g % 4][:, k * 2 + b : k * 2 + b + 1],
                        )
                for k in range(1 if "stt" in ablate else K):
                    for b in range(2):
                        vector.scalar_tensor_tensor(
                            out=scr[:],
                            in0=G[g % GDEPTH][
                                :, k * ROW + b * C : k * ROW + (b + 1) * C
                            ],
                            scalar=1.0,
                            in1=G[g % GDEPTH][
                                :, k * ROW + b * C : k * ROW + (b + 1) * C
                            ],
                            op0=OP.mult,
                            op1=OP.mult,
                            accum_out=nn_g[g % 2][:, k * 2 + b : k * 2 + b + 1],
                        )
                if g >= 1:
                    phase_nr(g - 1)  # lag: after this iter's 36 STTs
                if g >= 4:
                    phase_a2(g - 4)
            # tail (in-loop: b m<=n-4, nr m<=n-2, a1/a2 m<=n-5)
            nm = n_main
            spacer(3)
            phase_nr(nm - 1)
            phase_b(nm - 3)
            phase_a1(nm - 4)
            spacer(3)
            phase_a2(nm - 4)
            phase_b(nm - 2)
            phase_a1(nm - 3)
            spacer(3)
            phase_a2(nm - 3)
            phase_b(nm - 1)
            phase_a1(nm - 2)
            spacer(3)
            phase_a2(nm - 2)
            phase_a1(nm - 1)
            spacer(3)
            phase_a2(nm - 1)
            # epilogue: PSUM -> SBUF with 1/T scaling
            vector.wait_ge(pe_done, n_main)
            vector.tensor_scalar_mul(
                out=out_sb[0:1, :], in0=acc[0:1, :], scalar1=1.0 / (t_loc * main_reps)
            )
            vector.tensor_scalar_mul(
                out=out_sb[32:33, :],
                in0=acc[32:33, :],
                scalar1=1.0 / (t_loc * main_reps),
            ).then_inc(vfin, 1)

        @block.scalar
        def _(scalar: bass.BassScalarEngine):
            # ---- pre-pass: ninv = sqrt(1/nn) -> resident f32 ----
            for r in range(pre_reps):
                scalar.wait_ge(vpre, r + 1)
                scalar.activation(
                    out=ninv_sb[:], in_=nrec_all[:], func=AF.Sqrt
                ).then_inc(spre, 1)
            # ---- main loop: neighbor rsqrt + exp with fused denominators;
            # the aexp inc rides a trailing dummy op so the accum lands first
            for m in range(n_main):
                scalar.wait_ge(nrdy, m + 1)
                scalar.activation(
                    out=nsqi[m % 2][:], in_=nrec_g[m % 2][:], func=AF.Sqrt
                ).then_inc(srdy, 1)
                scalar.wait_ge(v1, m + 1)
                if m >= 2:
                    scalar.wait_ge(v2, m - 1)  # ee/ss[m%2] free (a2 lag 3)
                s23 = k3(sim2[m % 2][:])
                e3 = k3(ee[m % 2][:])
                for b in range(2):
                    scalar.activation(
                        out=e3[:, :, b : b + 1],
                        in_=s23[:, :, b : b + 1],
                        func=AF.Exp,
                        accum_out=ss[m % 2][:, b : b + 1],
                    )
                scalar.activation(
                    out=dscr[:], in_=sim2[m % 2][:, 0:2], func=AF.Copy
                ).then_inc(aexp, 1)

        @block.tensor
        def _(tensor: bass.BassTensorEngine):
            for g in range(n_main):
                tensor.wait_ge(v2, g + 1)
                for k in range(1 if "pe" in ablate else K):
                    for b in range(2):
                        mm = tensor.matmul(
                            out=acc[32 * b : 32 * b + 1, :],
                            lhsT=ww[g % 4][:, k * 2 + b : k * 2 + b + 1],
                            rhs=G[g % GDEPTH][
                                :, k * ROW + b * C : k * ROW + (b + 1) * C
                            ],
                            start=(g == 0 and k == 0),
                            stop=(g == n_main - 1 and k == K - 1),
                            skip_group_check=True,
                        )
                mm.then_inc(pe_done, 1)

        @block.gpsimd
        def _(gpsimd: bass.BassGpSimd):
            gpsimd.load_library(mlp)
            gpsimd.dma_start(out=idx_sb[:], in_=idx_ext[:]).then_inc(isem, 16)
            # ---- pre-pass: cast-DMA X f32 -> tab_sb bf16, chunked ----
            for r in range(pre_reps):
                for c in range(nch):
                    if r > 0:
                        # previous rep's consumers of this chunk must finish
                        gpsimd.wait_ge(vchk, (r - 1) * nch + c + 1)
                        gpsimd.wait_ge(tdone, 16 * ((r - 1) * nch + c + 1))
                    rows = slice(c * TPC * P, (c + 1) * TPC * P)
                    tv = tab_sb[
                        :, c * TPC * ROW : (c + 1) * TPC * ROW
                    ].rearrange("p (g b c) -> p g b c", b=2, c=C)
                    for b in range(2):
                        gpsimd.dma_start(
                            out=tv[:, :, b, :],
                            in_=x_ext[b, rows, :].rearrange(
                                "(g p) c -> p g c", p=P
                            ),
                        ).then_inc(xload, 16)
            # ---- main loop: gathers (after full table resident in DRAM) ----
            gpsimd.wait_ge(isem, 16)
            gpsimd.wait_ge(tdone, 16 * pre_reps * nch)
            for g in range(n_main):
                t = g % n_tiles
                if g >= GDEPTH:
                    gpsimd.wait_ge(pe_done, g - (GDEPTH - 2))  # G[g%GDEPTH] free
                gv3 = G[g % GDEPTH][:].rearrange("p (g r) -> p g r", r=ROW)
                spans = ((0, 1), (1, 2)) if "gather" in ablate else QSPANS
                for q, (g0, g1) in enumerate(spans):
                    n = (g1 - g0) * P
                    gpsimd.dma_gather(
                        gv3[:, g0:g1, :],
                        table[:],
                        idx_sb[
                            :,
                            t * IDX_COLS + g0 * 8 : t * IDX_COLS + g1 * 8,
                        ],
                        n,
                        n,
                        ROW,
                        single_packet=True,
                        queue_num=q % NQ,
                    ).then_inc(gsem[g % GDEPTH], 16)

    nc.compile()
    return nc


def make_idx_table(neighbor_idx: np.ndarray, n_tiles: int = NT) -> np.ndarray:
    """Host-side index preprocessing into dma_gather's wrapped int16 layout.

    Flat order per tile: i = k*128 + p  ->  neighbor_idx[t0+p, k].
    Wrapped: idx_sb[q, tile*IDX_COLS + c] = flat[c*16 + q%16].
    """
    nb = np.asarray(neighbor_idx).astype(np.int16)  # values < 4096
    cols = np.empty((P, n_tiles * IDX_COLS), dtype=np.int16)
    for t in range(n_tiles):
        flat = nb[t * P : (t + 1) * P, :].T.reshape(-1)  # [K*P], k-major
        wrap = flat.reshape(IDX_COLS, 16).T  # [16, IDX_COLS]
        cols[:, t * IDX_COLS : (t + 1) * IDX_COLS] = np.tile(wrap, (8, 1))
    return cols


_NC_CACHE: dict = {}


def _get_nc():
    if "nc" not in _NC_CACHE:
        _NC_CACHE["nc"] = build_kernel()
    return _NC_CACHE["nc"]


def kernel(X: np.ndarray, neighbor_idx: np.ndarray, **_ignored) -> np.ndarray:
    X = np.asarray(X, dtype=np.float32)
    idx_cols = make_idx_table(neighbor_idx)
    nc = _get_nc()
    core_ids = list(range(N_CORES))
    in_maps = [
        {"X": np.ascontiguousarray(X[i * B_LOC : (i + 1) * B_LOC]), "idx": idx_cols}
        for i in core_ids
    ]
    res = run_bass_kernel_spmd(nc, in_maps, core_ids)
    outs = [res.results[i]["out"] for i in range(N_CORES)]  # each [B_LOC, C]
    full = np.concatenate(outs, axis=0).reshape(B, 1, C).astype(np.float32)
    return full


if __name__ == "__main__":
    rng = np.random.default_rng(0)
    X = rng.standard_normal((B, T, C), dtype=np.float32)
    nb = rng.integers(0, T, size=(T, K)).astype(np.int64)
    out = kernel(X, nb)
    print("out", out.shape, out.dtype, float(np.abs(out).mean()))
